# revision 26
# baseline (speedup 1.0000x reference)
"""Segment mean-pool (AspireConSent) Trainium2 kernel.

Computes, for hidden [B=64, S=512, D=768] f32 and sent_ids [B, S] int32 in
[0, 20]:
  doc_cls_reps = hidden[:, 0, :]                                  [B, D]
  sent_reps[b, m] = mean over tokens s with sent_ids[b, s] == m   [B, 20, D]
with empty-sentence means = 0 (count clamped to >= 1).

Strategy: data parallel over 8 NeuronCores (8 examples per core). On each
core the segment-sum is a one-hot matmul on the TensorEngine:
  O[s, m] = (sent_ids[s] == m)  ->  sums = O.T @ H,  counts = O.T @ ones.
"""

import sys

import numpy as np

for _p in ("/opt/trn_rl_repo", "/root/.axon_site/_ro/trn_rl_repo"):
    if _p not in sys.path:
        sys.path.append(_p)

import concourse.bass as bass
import concourse.bacc as bacc
import concourse.mybir as mybir
from concourse.bass_utils import run_bass_kernel_spmd
from concourse.tile import TileContext

N_CORES = 8
B, S, D = 64, 512, 768
BL = B // N_CORES  # examples per core
MS = 20            # real sentence buckets
M1 = MS + 1        # + the "no sentence" bucket
P = 128
NCH = S // P       # 128-token chunks per example

_CACHE = {}


def build_nc() -> bass.Bass:
    f32 = mybir.dt.float32
    i32 = mybir.dt.int32

    nc = bacc.Bacc()
    hidden = nc.declare_dram_parameter("hidden", [BL, S, D], f32, isOutput=False)
    sent_ids = nc.declare_dram_parameter("sent_ids", [BL, S], i32, isOutput=False)
    out_cls = nc.declare_dram_parameter("out_cls", [BL, D], f32, isOutput=True)
    out_sent = nc.declare_dram_parameter("out_sent", [BL, MS, D], f32, isOutput=True)

    bf16 = mybir.dt.bfloat16
    DP = D + 1  # 768 hidden cols + 1 ones column (for counts)

    with TileContext(nc) as tc:
        with (
            tc.tile_pool(name="const", bufs=1) as cpool,
            tc.tile_pool(name="h", bufs=8) as hpool,
            tc.tile_pool(name="outp", bufs=3) as opool,
            tc.tile_pool(name="small", bufs=3) as spool,
            tc.tile_pool(name="psum_s", bufs=4, space="PSUM") as pspool,
        ):
            # ---- hidden loads first: SWDGE casts f32->bf16 inline, loads
            # complete FIFO at full BW so example b's compute can start as
            # soon as its data lands. Early/late examples are split into
            # smaller pieces to shorten the pipeline head/tail.
            split = {0: [1, 3]}  # chunk counts per DMA piece
            hts = []
            for b in range(BL):
                ht = hpool.tile([P, NCH, DP], bf16)
                hts.append(ht)
                pieces = split.get(b, [NCH])
                src = hidden[b].rearrange("(c p) d -> p c d", p=P)
                c0 = 0
                for cs in pieces:
                    nc.gpsimd.dma_start(out=ht[:, c0:c0 + cs, 0:D],
                                        in_=src[:, c0:c0 + cs, :])
                    c0 += cs
                nc.gpsimd.memset(ht[:, :, D:DP], 1.0)
                if b == 0:
                    # setup rides Q7 once the head load is in flight
                    iota_i = cpool.tile([P, M1], i32)
                    nc.gpsimd.iota(iota_i[:], pattern=[[1, M1]], base=0, channel_multiplier=0)

            # cls rows don't depend on anything: DRAM->DRAM on the Sync ring
            for b in range(BL):
                nc.sync.dma_start(out=out_cls[b], in_=hidden[b, 0, :])

            # ---- one-hot setup (Sync + DVE, overlaps the loads) ----
            ids_i = cpool.tile([BL * NCH, P], i32)  # [32, 128]
            nc.sync.dma_start(out=ids_i[:], in_=sent_ids.rearrange("b (c p) -> (b c) p", p=P))
            iota_f = cpool.tile([P, M1], f32)
            nc.vector.tensor_copy(iota_f[:], iota_i[:])
            ids_f = cpool.tile([BL * NCH, P], f32)
            nc.vector.tensor_copy(ids_f[:], ids_i[:])
            # transpose [32, 128] -> [128, 32] via 32x32 DVE blocks
            ids_t = cpool.tile([P, BL * NCH], f32)
            for j in range(P // 32):
                nc.vector.transpose(out=ids_t[32 * j:32 * (j + 1), :], in_=ids_f[:, 32 * j:32 * (j + 1)])

            # onehot[p, k, m] = (ids_t[p, k] == m), all 32 chunks in one op
            onehot = cpool.tile([P, BL * NCH, M1], bf16)
            nc.vector.tensor_tensor(
                out=onehot[:],
                in0=iota_f[:, None, :].to_broadcast([P, BL * NCH, M1]),
                in1=ids_t[:, :, None].to_broadcast([P, BL * NCH, M1]),
                op=mybir.AluOpType.is_equal,
            )

            # ---- per-example compute ----
            for b in range(BL):
                ht = hts[b]
                psum_s = pspool.tile([M1, DP], f32)

                def oh(c):
                    return onehot[:, b * NCH + c, :]

                # group with the counts column first so recip overlaps the
                # second group (shortens the last example's tail)
                for c in range(NCH):
                    nc.tensor.matmul(psum_s[:, 512:DP], oh(c), ht[:, c, 512:DP],
                                     start=(c == 0), stop=(c == NCH - 1))
                for c in range(NCH):
                    nc.tensor.matmul(psum_s[:, 0:512], oh(c), ht[:, c, 0:512],
                                     start=(c == 0), stop=(c == NCH - 1))

                # counts live in psum_s[:, 768]; mean = sums * (1/max(cnt,1))
                cnt = spool.tile([M1, 1], f32)
                nc.vector.tensor_scalar_max(cnt[:], psum_s[:, D:DP], 1.0)
                recip = spool.tile([M1, 1], f32)
                nc.vector.reciprocal(recip[:], cnt[:])

                outt = opool.tile([MS, D], f32)
                if b == BL - 1:
                    # split the final scale+store so the [512:768] half
                    # overlaps the last matmul group
                    nc.scalar.activation(
                        out=outt[:, 512:D], in_=psum_s[0:MS, 512:D],
                        func=mybir.ActivationFunctionType.Copy,
                        scale=recip[0:MS, :],
                    )
                    nc.scalar.dma_start(out=out_sent[b, :, 512:D], in_=outt[:, 512:D])
                    nc.scalar.activation(
                        out=outt[:, 0:512], in_=psum_s[0:MS, 0:512],
                        func=mybir.ActivationFunctionType.Copy,
                        scale=recip[0:MS, :],
                    )
                    nc.scalar.dma_start(out=out_sent[b, :, 0:512], in_=outt[:, 0:512])
                else:
                    nc.scalar.activation(
                        out=outt[:],
                        in_=psum_s[0:MS, 0:D],
                        func=mybir.ActivationFunctionType.Copy,
                        scale=recip[0:MS, :],
                    )
                    nc.scalar.dma_start(out=out_sent[b], in_=outt[:])
    nc.finalize()
    return nc


def _get_nc() -> bass.Bass:
    if "nc" not in _CACHE:
        _CACHE["nc"] = build_nc()
    return _CACHE["nc"]


def run(hidden, sent_ids, **spmd_kwargs):
    """Shard, run on 8 cores, gather. Returns (results_obj, cls, sent)."""
    hidden = np.ascontiguousarray(hidden, dtype=np.float32)
    sent_ids = np.ascontiguousarray(sent_ids, dtype=np.int32)
    nc = _get_nc()
    in_maps = [
        {
            "hidden": hidden[i * BL:(i + 1) * BL],
            "sent_ids": sent_ids[i * BL:(i + 1) * BL],
        }
        for i in range(N_CORES)
    ]
    res = run_bass_kernel_spmd(nc, in_maps, core_ids=list(range(N_CORES)), **spmd_kwargs)
    cls = np.concatenate([res.results[i]["out_cls"] for i in range(N_CORES)], axis=0)
    sent = np.concatenate([res.results[i]["out_sent"] for i in range(N_CORES)], axis=0)
    return res, cls, sent


def kernel(hidden, sent_ids, max_sents=20):
    assert int(max_sents) == MS
    _, cls, sent = run(hidden, sent_ids)
    return cls.astype(np.float32), sent.astype(np.float32)


if __name__ == "__main__":
    rng = np.random.default_rng(0)
    h = rng.standard_normal((B, S, D), dtype=np.float32)
    ids = rng.integers(0, M1, size=(B, S)).astype(np.int32)
    cls, sent = kernel(h, ids, MS)
    print("cls", cls.shape, "sent", sent.shape)


# revision 28
# speedup vs baseline: 1.0315x; 1.0315x over previous
"""Segment mean-pool (AspireConSent) Trainium2 kernel.

Computes, for hidden [B=64, S=512, D=768] f32 and sent_ids [B, S] int32 in
[0, 20]:
  doc_cls_reps = hidden[:, 0, :]                                  [B, D]
  sent_reps[b, m] = mean over tokens s with sent_ids[b, s] == m   [B, 20, D]
with empty-sentence means = 0 (count clamped to >= 1).

Strategy: data parallel over 8 NeuronCores (8 examples per core). On each
core the segment-sum is a one-hot matmul on the TensorEngine:
  O[s, m] = (sent_ids[s] == m)  ->  sums = O.T @ H,  counts = O.T @ ones.
"""

import sys

import numpy as np

for _p in ("/opt/trn_rl_repo", "/root/.axon_site/_ro/trn_rl_repo"):
    if _p not in sys.path:
        sys.path.append(_p)

import concourse.bass as bass
import concourse.bacc as bacc
import concourse.mybir as mybir
from concourse.bass_utils import run_bass_kernel_spmd
from concourse.tile import TileContext

N_CORES = 8
B, S, D = 64, 512, 768
BL = B // N_CORES  # examples per core
MS = 20            # real sentence buckets
M1 = MS + 1        # + the "no sentence" bucket
P = 128
NCH = S // P       # 128-token chunks per example

_CACHE = {}


def build_nc() -> bass.Bass:
    f32 = mybir.dt.float32
    i32 = mybir.dt.int32

    nc = bacc.Bacc()
    hidden = nc.declare_dram_parameter("hidden", [BL, S, D], f32, isOutput=False)
    sent_ids = nc.declare_dram_parameter("sent_ids", [BL, S], i32, isOutput=False)
    out_cls = nc.declare_dram_parameter("out_cls", [BL, D], f32, isOutput=True)
    out_sent = nc.declare_dram_parameter("out_sent", [BL, MS, D], f32, isOutput=True)

    bf16 = mybir.dt.bfloat16
    DP = D + 1  # 768 hidden cols + 1 ones column (for counts)

    with TileContext(nc) as tc:
        with (
            tc.tile_pool(name="const", bufs=1) as cpool,
            tc.tile_pool(name="h", bufs=8) as hpool,
            tc.tile_pool(name="outp", bufs=3) as opool,
            tc.tile_pool(name="small", bufs=3) as spool,
            tc.tile_pool(name="psum_s", bufs=4, space="PSUM") as pspool,
        ):
            # ---- head-critical Sync-ring loads: ids (gates one-hot build)
            # and b0's first chunk as f32 (HWDGE first-byte ~0.6us vs
            # SWDGE ~2us), cast to bf16 on DVE.
            ids_i = cpool.tile([BL * NCH, P], i32)  # [32, 128]
            nc.sync.dma_start(out=ids_i[:], in_=sent_ids.rearrange("b (c p) -> (b c) p", p=P))
            hf0 = cpool.tile([P, D], f32)
            nc.sync.dma_start(out=hf0[:], in_=hidden[0, 0:P, :])

            # ---- bulk hidden loads: SWDGE casts f32->bf16 inline, FIFO at
            # full BW so example b's compute starts as soon as its data lands.
            split = {0: [3]}  # b0 chunk 0 comes via the Sync ring above
            hts = []
            for b in range(BL):
                ht = hpool.tile([P, NCH, DP], bf16)
                hts.append(ht)
                pieces = split.get(b, [NCH])
                src = hidden[b].rearrange("(c p) d -> p c d", p=P)
                c0 = NCH - sum(pieces)
                for cs in pieces:
                    nc.gpsimd.dma_start(out=ht[:, c0:c0 + cs, 0:D],
                                        in_=src[:, c0:c0 + cs, :])
                    c0 += cs
                nc.gpsimd.memset(ht[:, :, D:DP], 1.0)
                if b == 0:
                    # setup rides Q7 once the head load is in flight
                    iota_i = cpool.tile([P, M1], i32)
                    nc.gpsimd.iota(iota_i[:], pattern=[[1, M1]], base=0, channel_multiplier=0)

            # cls rows don't depend on anything: DRAM->DRAM on the Sync ring
            for b in range(BL):
                nc.sync.dma_start(out=out_cls[b], in_=hidden[b, 0, :])

            # ---- one-hot build, all in int32 until the final compare ----
            # transpose [32, 128] -> [128, 32] via 32x32 DVE blocks
            ids_t = cpool.tile([P, BL * NCH], i32)
            for j in range(P // 32):
                nc.vector.transpose(out=ids_t[32 * j:32 * (j + 1), :], in_=ids_i[:, 32 * j:32 * (j + 1)])

            # b0 chunk 0: f32 -> bf16 cast (DVE, ~0.3us)
            nc.vector.tensor_copy(hts[0][:, 0, 0:D], hf0[:])

            # onehot[p, k, m] = (ids_t[p, k] == m), all 32 chunks in one op
            onehot = cpool.tile([P, BL * NCH, M1], bf16)
            nc.vector.tensor_tensor(
                out=onehot[:],
                in0=iota_i[:, None, :].to_broadcast([P, BL * NCH, M1]),
                in1=ids_t[:, :, None].to_broadcast([P, BL * NCH, M1]),
                op=mybir.AluOpType.is_equal,
            )

            # ---- per-example compute ----
            for b in range(BL):
                ht = hts[b]
                psum_s = pspool.tile([M1, DP], f32)

                def oh(c):
                    return onehot[:, b * NCH + c, :]

                # group with the counts column first so recip overlaps the
                # second group (shortens the last example's tail)
                for c in range(NCH):
                    nc.tensor.matmul(psum_s[:, 512:DP], oh(c), ht[:, c, 512:DP],
                                     start=(c == 0), stop=(c == NCH - 1))
                for c in range(NCH):
                    nc.tensor.matmul(psum_s[:, 0:512], oh(c), ht[:, c, 0:512],
                                     start=(c == 0), stop=(c == NCH - 1))

                # counts live in psum_s[:, 768]; mean = sums * (1/max(cnt,1))
                cnt = spool.tile([M1, 1], f32)
                nc.vector.tensor_scalar_max(cnt[:], psum_s[:, D:DP], 1.0)
                recip = spool.tile([M1, 1], f32)
                nc.vector.reciprocal(recip[:], cnt[:])

                outt = opool.tile([MS, D], f32)
                if b == BL - 1:
                    # split the final scale+store so the [512:768] half
                    # overlaps the last matmul group
                    nc.scalar.activation(
                        out=outt[:, 512:D], in_=psum_s[0:MS, 512:D],
                        func=mybir.ActivationFunctionType.Copy,
                        scale=recip[0:MS, :],
                    )
                    nc.sync.dma_start(out=out_sent[b, :, 512:D], in_=outt[:, 512:D])
                    nc.scalar.activation(
                        out=outt[:, 0:512], in_=psum_s[0:MS, 0:512],
                        func=mybir.ActivationFunctionType.Copy,
                        scale=recip[0:MS, :],
                    )
                    nc.sync.dma_start(out=out_sent[b, :, 0:512], in_=outt[:, 0:512])
                else:
                    nc.scalar.activation(
                        out=outt[:],
                        in_=psum_s[0:MS, 0:D],
                        func=mybir.ActivationFunctionType.Copy,
                        scale=recip[0:MS, :],
                    )
                    nc.sync.dma_start(out=out_sent[b], in_=outt[:])
    nc.finalize()
    return nc


def _get_nc() -> bass.Bass:
    if "nc" not in _CACHE:
        _CACHE["nc"] = build_nc()
    return _CACHE["nc"]


def run(hidden, sent_ids, **spmd_kwargs):
    """Shard, run on 8 cores, gather. Returns (results_obj, cls, sent)."""
    hidden = np.ascontiguousarray(hidden, dtype=np.float32)
    sent_ids = np.ascontiguousarray(sent_ids, dtype=np.int32)
    nc = _get_nc()
    in_maps = [
        {
            "hidden": hidden[i * BL:(i + 1) * BL],
            "sent_ids": sent_ids[i * BL:(i + 1) * BL],
        }
        for i in range(N_CORES)
    ]
    res = run_bass_kernel_spmd(nc, in_maps, core_ids=list(range(N_CORES)), **spmd_kwargs)
    cls = np.concatenate([res.results[i]["out_cls"] for i in range(N_CORES)], axis=0)
    sent = np.concatenate([res.results[i]["out_sent"] for i in range(N_CORES)], axis=0)
    return res, cls, sent


def kernel(hidden, sent_ids, max_sents=20):
    assert int(max_sents) == MS
    _, cls, sent = run(hidden, sent_ids)
    return cls.astype(np.float32), sent.astype(np.float32)


if __name__ == "__main__":
    rng = np.random.default_rng(0)
    h = rng.standard_normal((B, S, D), dtype=np.float32)
    ids = rng.integers(0, M1, size=(B, S)).astype(np.int32)
    cls, sent = kernel(h, ids, MS)
    print("cls", cls.shape, "sent", sent.shape)


# revision 29
# speedup vs baseline: 1.1530x; 1.1178x over previous
"""Segment mean-pool (AspireConSent) Trainium2 kernel.

Computes, for hidden [B=64, S=512, D=768] f32 and sent_ids [B, S] int32 in
[0, 20]:
  doc_cls_reps = hidden[:, 0, :]                                  [B, D]
  sent_reps[b, m] = mean over tokens s with sent_ids[b, s] == m   [B, 20, D]
with empty-sentence means = 0 (count clamped to >= 1).

Strategy: data parallel over 8 NeuronCores (8 examples per core). On each
core the segment-sum is a one-hot matmul on the TensorEngine:
  O[s, m] = (sent_ids[s] == m)  ->  sums = O.T @ H,  counts = O.T @ ones.
"""

import sys

import numpy as np

for _p in ("/opt/trn_rl_repo", "/root/.axon_site/_ro/trn_rl_repo"):
    if _p not in sys.path:
        sys.path.append(_p)

import concourse.bass as bass
import concourse.bacc as bacc
import concourse.mybir as mybir
from concourse.bass_utils import run_bass_kernel_spmd
from concourse.tile import TileContext

N_CORES = 8
B, S, D = 64, 512, 768
BL = B // N_CORES  # examples per core
MS = 20            # real sentence buckets
M1 = MS + 1        # + the "no sentence" bucket
P = 128
NCH = S // P       # 128-token chunks per example

_CACHE = {}


def build_nc() -> bass.Bass:
    f32 = mybir.dt.float32
    i32 = mybir.dt.int32

    nc = bacc.Bacc()
    hidden = nc.declare_dram_parameter("hidden", [BL, S, D], f32, isOutput=False)
    sent_ids = nc.declare_dram_parameter("sent_ids", [BL, S], i32, isOutput=False)
    out_cls = nc.declare_dram_parameter("out_cls", [BL, D], f32, isOutput=True)
    out_sent = nc.declare_dram_parameter("out_sent", [BL, MS, D], f32, isOutput=True)

    bf16 = mybir.dt.bfloat16
    DP = D + 1  # 768 hidden cols + 1 ones column (for counts)

    with TileContext(nc) as tc:
        with (
            tc.tile_pool(name="const", bufs=1) as cpool,
            tc.tile_pool(name="h", bufs=8) as hpool,
            tc.tile_pool(name="outp", bufs=3) as opool,
            tc.tile_pool(name="small", bufs=3) as spool,
            tc.tile_pool(name="psum_s", bufs=4, space="PSUM") as pspool,
        ):
            # ---- head-critical Sync-ring loads: ids (gates one-hot build)
            # and b0's first chunk as f32 (HWDGE first-byte ~0.6us vs
            # SWDGE ~2us), cast to bf16 on DVE.
            ids_i = cpool.tile([BL * NCH, P], i32)  # [32, 128]
            nc.sync.dma_start(out=ids_i[:], in_=sent_ids.rearrange("b (c p) -> (b c) p", p=P))
            hf0 = cpool.tile([P, D], f32)
            nc.sync.dma_start(out=hf0[:], in_=hidden[0, 0:P, :])

            # ---- bulk hidden loads: SWDGE casts f32->bf16 inline, FIFO at
            # full BW so example b's compute starts as soon as its data lands.
            split = {0: [3], BL - 1: [3, 1]}  # b0 chunk 0 comes via the Sync ring
            hts = []
            for b in range(BL):
                ht = hpool.tile([P, NCH, DP], bf16)
                hts.append(ht)
                pieces = split.get(b, [NCH])
                src = hidden[b].rearrange("(c p) d -> p c d", p=P)
                c0 = NCH - sum(pieces)
                for cs in pieces:
                    nc.gpsimd.dma_start(out=ht[:, c0:c0 + cs, 0:D],
                                        in_=src[:, c0:c0 + cs, :])
                    c0 += cs
                nc.gpsimd.memset(ht[:, :, D:DP], 1.0)
                if b == 0:
                    # setup rides Q7 once the head load is in flight
                    iota_i = cpool.tile([P, M1], i32)
                    nc.gpsimd.iota(iota_i[:], pattern=[[1, M1]], base=0, channel_multiplier=0)

            # cls rows don't depend on anything: DRAM->DRAM on the Sync ring
            for b in range(BL):
                nc.sync.dma_start(out=out_cls[b], in_=hidden[b, 0, :])

            # ---- one-hot build, all in int32 until the final compare ----
            # transpose [32, 128] -> [128, 32] via 32x32 DVE blocks
            ids_t = cpool.tile([P, BL * NCH], i32)
            for j in range(P // 32):
                nc.vector.transpose(out=ids_t[32 * j:32 * (j + 1), :], in_=ids_i[:, 32 * j:32 * (j + 1)])

            # b0 chunk 0: f32 -> bf16 cast (ACT, parallel to the DVE chain)
            nc.scalar.copy(hts[0][:, 0, 0:D], hf0[:])

            # onehot[p, k, m] = (ids_t[p, k] == m), all 32 chunks in one op
            onehot = cpool.tile([P, BL * NCH, M1], bf16)
            nc.vector.tensor_tensor(
                out=onehot[:],
                in0=iota_i[:, None, :].to_broadcast([P, BL * NCH, M1]),
                in1=ids_t[:, :, None].to_broadcast([P, BL * NCH, M1]),
                op=mybir.AluOpType.is_equal,
            )

            # ---- per-example compute ----
            for b in range(BL):
                ht = hts[b]
                psum_s = pspool.tile([M1, DP], f32)

                def oh(c):
                    return onehot[:, b * NCH + c, :]

                # interleave the two accumulation groups per chunk so the
                # in-order PE never stalls a whole group on the last-arriving
                # chunk; counts-group member goes first per chunk.
                for c in range(NCH):
                    nc.tensor.matmul(psum_s[:, 512:DP], oh(c), ht[:, c, 512:DP],
                                     start=(c == 0), stop=(c == NCH - 1),
                                     skip_group_check=True)
                    nc.tensor.matmul(psum_s[:, 0:512], oh(c), ht[:, c, 0:512],
                                     start=(c == 0), stop=(c == NCH - 1),
                                     skip_group_check=True)

                # counts live in psum_s[:, 768]; mean = sums * (1/max(cnt,1))
                cnt = spool.tile([M1, 1], f32)
                nc.vector.tensor_scalar_max(cnt[:], psum_s[:, D:DP], 1.0)
                recip = spool.tile([M1, 1], f32)
                nc.vector.reciprocal(recip[:], cnt[:])

                outt = opool.tile([MS, D], f32)
                if b == BL - 1:
                    # split the final scale+store so the [512:768] half
                    # overlaps the last matmul group
                    nc.scalar.activation(
                        out=outt[:, 512:D], in_=psum_s[0:MS, 512:D],
                        func=mybir.ActivationFunctionType.Copy,
                        scale=recip[0:MS, :],
                    )
                    nc.sync.dma_start(out=out_sent[b, :, 512:D], in_=outt[:, 512:D])
                    nc.scalar.activation(
                        out=outt[:, 0:512], in_=psum_s[0:MS, 0:512],
                        func=mybir.ActivationFunctionType.Copy,
                        scale=recip[0:MS, :],
                    )
                    nc.sync.dma_start(out=out_sent[b, :, 0:512], in_=outt[:, 0:512])
                else:
                    nc.scalar.activation(
                        out=outt[:],
                        in_=psum_s[0:MS, 0:D],
                        func=mybir.ActivationFunctionType.Copy,
                        scale=recip[0:MS, :],
                    )
                    nc.sync.dma_start(out=out_sent[b], in_=outt[:])
    nc.finalize()
    return nc


def _get_nc() -> bass.Bass:
    if "nc" not in _CACHE:
        _CACHE["nc"] = build_nc()
    return _CACHE["nc"]


def run(hidden, sent_ids, **spmd_kwargs):
    """Shard, run on 8 cores, gather. Returns (results_obj, cls, sent)."""
    hidden = np.ascontiguousarray(hidden, dtype=np.float32)
    sent_ids = np.ascontiguousarray(sent_ids, dtype=np.int32)
    nc = _get_nc()
    in_maps = [
        {
            "hidden": hidden[i * BL:(i + 1) * BL],
            "sent_ids": sent_ids[i * BL:(i + 1) * BL],
        }
        for i in range(N_CORES)
    ]
    res = run_bass_kernel_spmd(nc, in_maps, core_ids=list(range(N_CORES)), **spmd_kwargs)
    cls = np.concatenate([res.results[i]["out_cls"] for i in range(N_CORES)], axis=0)
    sent = np.concatenate([res.results[i]["out_sent"] for i in range(N_CORES)], axis=0)
    return res, cls, sent


def kernel(hidden, sent_ids, max_sents=20):
    assert int(max_sents) == MS
    _, cls, sent = run(hidden, sent_ids)
    return cls.astype(np.float32), sent.astype(np.float32)


if __name__ == "__main__":
    rng = np.random.default_rng(0)
    h = rng.standard_normal((B, S, D), dtype=np.float32)
    ids = rng.integers(0, M1, size=(B, S)).astype(np.int32)
    cls, sent = kernel(h, ids, MS)
    print("cls", cls.shape, "sent", sent.shape)
